# revision 24
# baseline (speedup 1.0000x reference)
"""Trainium2 Bass kernel for nn_CustomConv_1417339208184 (GNN message passing).

Strategy (matches the sharding hint): shard the 131072 interferes edges
across the 8 cores by ap (each core owns 16 aps = 16384 edges).  The
structured indices (int_src = repeat(arange(128), 1024), int_dst =
tile(arange(1024), 128)) let the gather fold into a per-ap bias and the
segment-sum fold into PSUM accumulation.  One AllReduce mid-kernel merges
the per-core partial ue states ([64,1024] mlp2 segment sums + the by-u
sharded d_link interference partials); the final interference aggregate is
computed per-core against SBUF-resident P = |pl_int|^2 tiles and summed on
the host.

Host side does layout only (transpose/slice/stack of inputs, transpose of
outputs, final 16-row partial sum); all O(E) math runs on device.
"""

import numpy as np
from concourse import bacc, bass, mybir, tile
from concourse.bass_utils import run_bass_kernel_spmd

f32 = mybir.dt.float32
ActFn = mybir.ActivationFunctionType
AluOp = mybir.AluOpType
DynSlice = bass.DynSlice

A, U, ANT, D2, H, NUE = 128, 8, 64, 64, 256, 1024
E_INT = A * NUE          # 131072
N_CORES = 8
APC = A // N_CORES       # 16 aps per core
EC = APC * NUE           # 16384 edges per core
BLK = 512                # edges per inner block
NBLK = EC // BLK         # 32
NPAIR = APC // 2         # 8 P-tile ap pairs per core

X = mybir.AxisListType.X

_PROG_CACHE = {}


# --------------------------------------------------------------------------
# Device program
# --------------------------------------------------------------------------

def _build_program():
    nc = bacc.Bacc("TRN2", target_bir_lowering=False, debug=False,
                   num_devices=N_CORES)

    def din(name, shape):
        return nc.dram_tensor(name, shape, f32, kind="ExternalInput").ap()

    # Per-core sharded input
    xt = din("xt", [128, EC])              # [pl_re.T ; pl_im.T] slab, local aps
    # Replicated inputs
    pvlocT = din("pvlocT", [128, APC * U])  # local pv.T: [re|im ant, (a_loc,u)]
    pvu_re = din("pvu_re", [A, ANT])       # pv[:, rank, :] (host-sliced)
    pvu_im = din("pvu_im", [A, ANT])
    mask_u = din("mask_u", [A, U])         # ones, column `rank` zeroed
    pldl_re = din("pldl_re", [A, U * ANT])  # pl_dl natural [a, (e', ant)]
    pldl_im = din("pldl_im", [A, U * ANT])
    pldlT = din("pldlT", [128, NUE])       # [pl_dl_re.T ; pl_dl_im.T]
    wc = din("wc", [128, H])               # [W2a[0:64] ; W2a[128:192]]
    ws = din("ws", [128, H])               # [W2a[64:128] ; W2a[192:256]]
    w2b = din("w2b", [H, D2])
    b2a_c = din("b2a_c", [H, 1])
    b2b128_c = din("b2b128_c", [D2, 1])    # 128 * b2b
    w1a_k0 = din("w1a_k0", [128, H])       # reordered W1a rows (see host prep)
    w1a_k1 = din("w1a_k1", [65, H])
    b1a_c = din("b1a_c", [H, 1])
    w1b = din("w1b", [H, 2 * ANT])
    b1b_c = din("b1b_c", [2 * ANT, 1])
    i2col = din("i2col", [128, ANT])       # vstack(I64, I64)
    i2x2 = din("i2x2", [128, 128])         # tile(I64, (2, 2))

    pvnewT_out = nc.dram_tensor("pvnewT_out", [2, ANT, NUE], f32,
                                kind="ExternalOutput").ap()
    ueint_out = nc.dram_tensor("ueint_out", [1, NUE], f32,
                               kind="ExternalOutput").ap()

    with tile.TileContext(nc) as tc:
        _emit(tc, locals())
    nc.compile()
    return nc


def _emit(tc, t):
    nc = tc.nc

    const_cm = tc.tile_pool(name="const", bufs=1)
    work_cm = tc.tile_pool(name="work", bufs=1)
    pt_cm = tc.tile_pool(name="ptpool", bufs=1)
    dram_cm = tc.tile_pool(name="dram", bufs=1, space="DRAM")
    const = const_cm.__enter__()
    work = work_cm.__enter__()
    ptp = pt_cm.__enter__()
    dram = dram_cm.__enter__()

    # ---------------- constants / weights into SBUF ----------------
    def load(name, shape, src):
        tl = const.tile(shape, f32, tag=name, name=name + "_sb")
        nc.sync.dma_start(out=tl, in_=src)
        return tl

    wc_sb = load("wc", [128, H], t["wc"])
    ws_sb = load("ws", [128, H], t["ws"])
    w2b_sb = load("w2b", [128, 2, D2],
                  t["w2b"].rearrange("(c p) d -> p c d", p=128))
    b2a_sb = load("b2a", [128, 2],
                  t["b2a_c"].rearrange("(c p) o -> p (c o)", p=128))
    b2b128_sb = load("b2b128", [D2, 1], t["b2b128_c"])
    w1a0_sb = load("w1a0", [128, H], t["w1a_k0"])
    w1a1_sb = load("w1a1", [65, H], t["w1a_k1"])
    b1a_sb = load("b1a", [128, 2],
                  t["b1a_c"].rearrange("(c p) o -> p (c o)", p=128))
    w1b_sb = load("w1b", [128, 2, 2 * ANT],
                  t["w1b"].rearrange("(c p) d -> p c d", p=128))
    b1b_sb = load("b1b", [128, 1], t["b1b_c"])
    pldlT_sb = load("pldlT", [128, NUE], t["pldlT"])
    pvlocT_sb = load("pvlocT", [128, APC * U], t["pvlocT"])
    pvu_re_sb = load("pvu_re", [A, ANT], t["pvu_re"])
    pvu_im_sb = load("pvu_im", [A, ANT], t["pvu_im"])
    mask_sb = load("mask_u", [A, U], t["mask_u"])
    i2col_sb = load("i2col", [128, ANT], t["i2col"])
    i2x2_sb = load("i2x2", [128, 128], t["i2x2"])

    cc_in = dram.tile([D2 + 1, NUE], f32, tag="cc_in")
    cc_out = dram.tile([D2 + 1, NUE], f32, tag="cc_out")

    # ---------------- B2 per-ap bias:  B2.T[h, a_loc] ----------------
    psA_cm = tc.tile_pool(name="psA", bufs=1, space="PSUM")
    psA = psA_cm.__enter__()

    sT_sb = const.tile([128, APC], f32, tag="sT")    # [re|im ant, a_loc]
    nc.vector.tensor_reduce(
        sT_sb, pvlocT_sb.rearrange("p (a u) -> p a u", u=U), X, AluOp.add)

    b2_sb = const.tile([128, 2, APC], f32, tag="b2sb")
    for m in range(2):
        pb = psA.tile([128, APC], f32, tag="pb2")
        nc.tensor.matmul(pb, lhsT=ws_sb[:, m * 128:(m + 1) * 128], rhs=sT_sb,
                         start=True, stop=True)
        nc.vector.tensor_scalar_add(b2_sb[:, m, :], pb, b2a_sb[:, m:m + 1])
    psA_cm.__exit__(None, None, None)

    # ---------------- stage 3 partial (this core's u = rank) --------------
    pldl_re_v = pldl_re_sb = load("pldl_re", [A, U * ANT], t["pldl_re"])
    pldl_im_v = pldl_im_sb = load("pldl_im", [A, U * ANT], t["pldl_im"])
    pldl_re_v = pldl_re_sb.rearrange("p (e a) -> p e a", e=U)
    pldl_im_v = pldl_im_sb.rearrange("p (e a) -> p e a", e=U)
    pvu_re_b = pvu_re_sb.unsqueeze(1).broadcast_to([A, U, ANT])
    pvu_im_b = pvu_im_sb.unsqueeze(1).broadcast_to([A, U, ANT])

    s3a = work.tile([A, U, ANT], f32, tag="s3a")
    s3b = work.tile([A, U, ANT], f32, tag="s3b")
    iner_re = work.tile([A, U], f32, tag="iner_re")
    iner_im = work.tile([A, U], f32, tag="iner_im")
    # iner_re = sum_ant pv_r*pl_r + pv_i*pl_i
    nc.vector.tensor_mul(s3a, pldl_re_v, pvu_re_b)
    nc.vector.tensor_mul(s3b, pldl_im_v, pvu_im_b)
    nc.vector.tensor_add(s3a, s3a, s3b)
    nc.vector.tensor_reduce(iner_re, s3a, X, AluOp.add)
    # iner_im = sum_ant pv_r*pl_i - pv_i*pl_r
    nc.vector.tensor_mul(s3a, pldl_im_v, pvu_re_b)
    nc.vector.tensor_mul(s3b, pldl_re_v, pvu_im_b)
    nc.vector.tensor_sub(s3a, s3a, s3b)
    nc.vector.tensor_reduce(iner_im, s3a, X, AluOp.add)

    norm = work.tile([A, U], f32, tag="norm")
    nc.vector.tensor_mul(norm, iner_re, iner_re)
    nc.vector.tensor_mul(iner_im, iner_im, iner_im)
    nc.vector.tensor_add(norm, norm, iner_im)
    nc.vector.tensor_mul(norm, norm, mask_sb)
    nc.sync.dma_start(
        out=cc_in[D2:D2 + 1, :].rearrange("o (a e) -> (o a) e", a=A),
        in_=norm)

    # ---------------- pass 1: edge MLP2 + P tiles ----------------
    psM_cm = tc.tile_pool(name="psM", bufs=1, space="PSUM")
    psM = psM_cm.__enter__()
    psH_cm = tc.tile_pool(name="psH", bufs=2, space="PSUM")
    psH = psH_cm.__enter__()

    pt_tiles = [ptp.tile([128, NUE], f32, tag=f"pt{p}", name=f"pt{p}")
                for p in range(APC)]
    macc = [psM.tile([D2, BLK], f32, tag=f"acc{h}", name=f"macc{h}")
            for h in range(2)]

    for b in range(NBLK):
        ap_loc, half = b // 2, b % 2
        xt_t = work.tile([128, BLK], f32, tag="xt", name="xt_t", bufs=3)
        nc.sync.dma_start(out=xt_t, in_=t["xt"][:, b * BLK:(b + 1) * BLK])

        rh = work.tile([128, 2, BLK], f32, tag="rh", name="rh", bufs=3)
        for m in range(2):
            ph = psH.tile([128, BLK], f32, tag=f"h{m}", name=f"ph{m}")
            nc.tensor.matmul(ph, lhsT=wc_sb[:, m * 128:(m + 1) * 128],
                             rhs=xt_t, start=True, stop=True)
            nc.scalar.activation(rh[:, m, :], ph, ActFn.Relu,
                                 bias=b2_sb[:, m, ap_loc:ap_loc + 1])
        for k in range(2):
            nc.tensor.matmul(macc[half], lhsT=w2b_sb[:, k, :], rhs=rh[:, k, :],
                             start=(b < 2 and k == 0),
                             stop=(b >= NBLK - 2 and k == 1))

        nc.vector.tensor_mul(
            pt_tiles[ap_loc][:, half * BLK:(half + 1) * BLK], xt_t, xt_t)

    msum_sb = work.tile([D2, NUE], f32, tag="msum")
    for half in range(2):
        nc.scalar.copy(msum_sb[:, half * BLK:(half + 1) * BLK], macc[half])
    nc.sync.dma_start(out=cc_in[0:D2, :], in_=msum_sb)

    # ---------------- AllReduce ----------------
    nc.gpsimd.collective_compute(
        "AllReduce", AluOp.add,
        replica_groups=[list(range(N_CORES))],
        ins=[cc_in[:, :]], outs=[cc_out[:, :]])

    f1b = work.tile([D2 + 1, NUE], f32, tag="f1b")
    nc.sync.dma_start(out=f1b, in_=cc_out[:, :])
    # + 128*b2b on the mlp_ue rows
    nc.vector.tensor_scalar_add(f1b[0:D2, :], f1b[0:D2, :], b2b128_sb[:, 0:1])

    # ---------------- MLP1 (replicated on every core) ----------------
    uepvT = work.tile([128, NUE], f32, tag="uepvT")   # rows 0:64 re, 64:128 im
    for eh in range(2):
        sl = slice(eh * BLK, (eh + 1) * BLK)
        rh1 = work.tile([128, 2, BLK], f32, tag="rh", name="rh1", bufs=3)
        for m in range(2):
            ph = psH.tile([128, BLK], f32, tag=f"h{m}", name=f"ph1{m}")
            nc.tensor.matmul(ph, lhsT=w1a0_sb[:, m * 128:(m + 1) * 128],
                             rhs=pldlT_sb[:, sl], start=True, stop=False)
            nc.tensor.matmul(ph, lhsT=w1a1_sb[:, m * 128:(m + 1) * 128],
                             rhs=f1b[:, sl], start=False, stop=True)
            nc.scalar.activation(rh1[:, m, :], ph, ActFn.Relu,
                                 bias=b1a_sb[:, m:m + 1])
        o1 = psM.tile([128, BLK], f32, tag=f"acc{eh}", name=f"o1{eh}")
        for k in range(2):
            nc.tensor.matmul(o1, lhsT=w1b_sb[:, k, :], rhs=rh1[:, k, :],
                             start=(k == 0), stop=(k == 1))
        nc.vector.tensor_scalar_add(uepvT[:, sl], o1, b1b_sb[:, 0:1])
    psH_cm.__exit__(None, None, None)

    # ---------------- stage 4: normalize ue power vectors ----------------
    psS_cm = tc.tile_pool(name="psS", bufs=1, space="PSUM")
    psS = psS_cm.__enter__()

    ones_col = const.tile([ANT, 1], f32, tag="ones_col")
    nc.vector.memset(ones_col, 1.0)
    ones_row = const.tile([1, 128], f32, tag="ones_row")
    nc.vector.memset(ones_row, 1.0)

    # |uepv|^2 summed over re/im via stacked-identity matmul fold
    sq4 = work.tile([128, NUE], f32, tag="sq4")
    nc.vector.tensor_mul(sq4, uepvT, uepvT)
    psqs = psS.tile([ANT, NUE], f32, tag="big", name="psqs")
    for hh in range(2):
        nc.tensor.matmul(psqs[:, hh * BLK:(hh + 1) * BLK], lhsT=i2col_sb,
                         rhs=sq4[:, hh * BLK:(hh + 1) * BLK],
                         start=True, stop=True)
    absab = work.tile([ANT, NUE], f32, tag="absab")
    nc.scalar.activation(absab, psqs, ActFn.Sqrt)

    rs8 = work.tile([1, NUE], f32, tag="rs8")
    for hh in range(2):
        prs = psS.tile([1, BLK], f32, tag=f"s{hh}", name=f"prs{hh}")
        nc.tensor.matmul(prs, lhsT=ones_col,
                         rhs=absab[:, hh * BLK:(hh + 1) * BLK],
                         start=True, stop=True)
        nc.vector.tensor_copy(rs8[:, hh * BLK:(hh + 1) * BLK], prs)
    rsa = work.tile([1, A], f32, tag="rsa")
    nc.vector.tensor_reduce(rsa, rs8.rearrange("p (a u) -> p a u", u=U),
                            X, AluOp.add)
    recip = work.tile([1, A], f32, tag="recip")
    nc.vector.reciprocal(recip, rsa)
    recip_au = work.tile([1, NUE], f32, tag="recip_au")
    rau_v = recip_au.rearrange("p (a u) -> p u a", u=U)
    for uu in range(U):
        nc.vector.tensor_copy(rau_v[:, uu, :], recip)

    # broadcast 1/row_sum to all 128 partitions
    bc2 = psS.tile([128, NUE], f32, tag="big", name="bc2")
    for hh in range(2):
        nc.tensor.matmul(bc2[:, hh * BLK:(hh + 1) * BLK], lhsT=ones_row,
                         rhs=recip_au[:, hh * BLK:(hh + 1) * BLK],
                         start=True, stop=True)

    pvn = work.tile([128, NUE], f32, tag="pvn")   # rows 0:64 re, 64:128 im
    nc.vector.tensor_mul(pvn, uepvT, bc2)
    for s in range(2):
        nc.sync.dma_start(out=t["pvnewT_out"][s],
                          in_=pvn[s * 64:(s + 1) * 64, :])

    # ---------------- final aggregate_interferes ----------------
    s2f = work.tile([128, A], f32, tag="s2f")     # sum over u, re|im stacked
    nc.vector.tensor_reduce(s2f, pvn.rearrange("p (a u) -> p a u", u=U),
                            X, AluOp.add)
    nc.vector.tensor_mul(s2f, s2f, s2f)
    # Q duplicated into both halves: rows 0:64 and 64:128 both = |s2|^2
    pqd = psS.tile([128, A], f32, tag="qtd", name="pqd")
    nc.tensor.matmul(pqd, lhsT=i2x2_sb, rhs=s2f, start=True, stop=True)

    qdup = const.tile([128, APC], f32, tag="qdup")
    rank = nc.vector.partition_id()
    nc.vector.tensor_copy(qdup, pqd[:, DynSlice(rank * APC, APC)])

    for hh in range(2):
        pue = psM.tile([1, BLK], f32, tag=f"acc{hh}", name=f"pue{hh}")
        for a_loc in range(APC):
            nc.tensor.matmul(pue, lhsT=qdup[:, a_loc:a_loc + 1],
                             rhs=pt_tiles[a_loc][:, hh * BLK:(hh + 1) * BLK],
                             start=(a_loc == 0), stop=(a_loc == APC - 1))
        sbue = work.tile([1, BLK], f32, tag="sbue", name=f"sbue{hh}")
        nc.vector.tensor_copy(sbue, pue)
        nc.sync.dma_start(out=t["ueint_out"][:, hh * BLK:(hh + 1) * BLK],
                          in_=sbue)

    psS_cm.__exit__(None, None, None)
    psM_cm.__exit__(None, None, None)
    dram_cm.__exit__(None, None, None)
    pt_cm.__exit__(None, None, None)
    work_cm.__exit__(None, None, None)
    const_cm.__exit__(None, None, None)


# --------------------------------------------------------------------------
# Host side
# --------------------------------------------------------------------------

def _f32(x):
    return np.ascontiguousarray(np.asarray(x, dtype=np.float32))


def _is_structured(inp):
    return (np.array_equal(inp["int_src"], np.repeat(np.arange(A), NUE)) and
            np.array_equal(inp["int_dst"], np.tile(np.arange(NUE), A)) and
            np.array_equal(inp["dl_src"], np.repeat(np.arange(A), U)) and
            np.array_equal(inp["dl_dst"], np.arange(NUE)))


def _reference_numpy(inp):
    """Exact numpy port of the reference — fallback for non-structured
    index inputs (never hit for the canonical setup_inputs)."""
    pv = inp["pv_re"] + 1j * inp["pv_im"]
    pl_int = inp["pl_int_re"] + 1j * inp["pl_int_im"]
    pl_dl = inp["pl_dl_re"] + 1j * inp["pl_dl_im"]
    int_src, int_dst = inp["int_src"], inp["int_dst"]
    dl_src, dl_dst = inp["dl_src"], inp["dl_dst"]

    def mlp(x, Wa, ba, Wb, bb):
        return np.maximum(x @ Wa + ba, 0.0) @ Wb + bb

    def seg_sum(vals, idx, n):
        out = np.zeros((n,) + vals.shape[1:], vals.dtype)
        np.add.at(out, idx, vals)
        return out

    src_sum = pv.sum(axis=1)[int_src]
    feats = np.concatenate([pl_int, src_sum], axis=1)
    cat = np.concatenate([feats.real, feats.imag], axis=1).astype(np.float32)
    msg2 = mlp(cat, inp["W2a"], inp["b2a"], inp["W2b"], inp["b2b"])
    mlp_ue = seg_sum(msg2, int_dst, NUE)

    src_pv = pv[dl_src]
    iner = np.einsum("eua,ea->eu", src_pv.conj(), pl_dl)
    norm1 = np.abs(iner) ** 2
    mask1 = np.ones((NUE, U), norm1.dtype)
    mask1[np.arange(NUE), dl_dst % U] = 0.0
    in_inf = (norm1 * mask1).sum(axis=1)[:, None].astype(np.float32)
    feat1 = np.concatenate([in_inf, pl_dl.real, pl_dl.imag,
                            mlp_ue[dl_dst]], axis=1).astype(np.float32)
    out1 = mlp(feat1, inp["W1a"], inp["b1a"], inp["W1b"],
               inp["b1b"]).reshape(NUE, 2, ANT)
    ue_msg = out1[:, 0, :] + 1j * out1[:, 1, :]
    ue_pv = np.zeros((NUE, ANT), ue_msg.dtype)
    ue_pv[dl_dst] = ue_msg

    flat = ue_pv.reshape(A, U * ANT)
    rs = np.abs(flat).sum(axis=-1, keepdims=True)
    pv_new = (flat / rs).reshape(A, U, ANT)

    s2 = pv_new.sum(axis=1)[int_src]
    infer = s2.conj() * pl_int
    sap = (np.abs(infer) ** 2).sum(axis=-1)
    ue_int = seg_sum(sap[:, None].astype(np.float32), int_dst, NUE)

    out_stacked = np.stack([pv_new.real, pv_new.imag]).astype(np.float32)
    return out_stacked, ue_int


def _make_in_maps(inp):
    pl_re = _f32(inp["pl_int_re"]).reshape(A, NUE, ANT)
    pl_im = _f32(inp["pl_int_im"]).reshape(A, NUE, ANT)
    pv_re3 = _f32(inp["pv_re"])                    # [A, U, ANT]
    pv_im3 = _f32(inp["pv_im"])
    pv_re = pv_re3.reshape(A, U * ANT)
    pv_im = pv_im3.reshape(A, U * ANT)
    pldl_re2 = _f32(inp["pl_dl_re"])               # [NUE, ANT]
    pldl_im2 = _f32(inp["pl_dl_im"])
    W2a, W2b = _f32(inp["W2a"]), _f32(inp["W2b"])
    W1a, W1b = _f32(inp["W1a"]), _f32(inp["W1b"])

    common = {
        "pldl_re": np.ascontiguousarray(pldl_re2.reshape(A, U * ANT)),
        "pldl_im": np.ascontiguousarray(pldl_im2.reshape(A, U * ANT)),
        "pldlT": np.ascontiguousarray(
            np.concatenate([pldl_re2.T, pldl_im2.T], axis=0)),
        "wc": np.ascontiguousarray(np.concatenate([W2a[0:64], W2a[128:192]])),
        "ws": np.ascontiguousarray(np.concatenate([W2a[64:128], W2a[192:256]])),
        "w2b": W2b,
        "b2a_c": _f32(inp["b2a"]).reshape(H, 1),
        "b2b128_c": _f32(inp["b2b"]).reshape(D2, 1) * np.float32(A),
        "w1a_k0": np.ascontiguousarray(
            np.concatenate([W1a[1:65], W1a[65:129]])),
        "w1a_k1": np.ascontiguousarray(
            np.concatenate([W1a[129:193], W1a[0:1]])),
        "b1a_c": _f32(inp["b1a"]).reshape(H, 1),
        "w1b": W1b,
        "b1b_c": _f32(inp["b1b"]).reshape(2 * ANT, 1),
        "i2col": np.ascontiguousarray(
            np.concatenate([np.eye(ANT, dtype=np.float32)] * 2, axis=0)),
        "i2x2": np.ascontiguousarray(
            np.tile(np.eye(ANT, dtype=np.float32), (2, 2))),
    }

    in_maps = []
    for c in range(N_CORES):
        a0 = c * APC
        xt_re = pl_re[a0:a0 + APC].transpose(2, 0, 1).reshape(ANT, EC)
        xt_im = pl_im[a0:a0 + APC].transpose(2, 0, 1).reshape(ANT, EC)
        mask = np.ones((A, U), np.float32)
        mask[:, c] = 0.0
        m = dict(common)
        m["xt"] = np.ascontiguousarray(
            np.concatenate([xt_re, xt_im], axis=0))
        m["pvlocT"] = np.ascontiguousarray(np.concatenate([
            pv_re[a0:a0 + APC].reshape(APC * U, ANT).T,
            pv_im[a0:a0 + APC].reshape(APC * U, ANT).T], axis=0))
        m["pvu_re"] = np.ascontiguousarray(pv_re3[:, c, :])
        m["pvu_im"] = np.ascontiguousarray(pv_im3[:, c, :])
        m["mask_u"] = mask
        in_maps.append(m)
    return in_maps


def _get_program():
    if "nc" not in _PROG_CACHE:
        _PROG_CACHE["nc"] = _build_program()
    return _PROG_CACHE["nc"]


def _run_device(inp, trace=False):
    nc = _get_program()
    in_maps = _make_in_maps(inp)
    res = run_bass_kernel_spmd(nc, in_maps, list(range(N_CORES)), trace=trace)
    r0 = res.results[0]
    pvnT = r0["pvnewT_out"]                  # [2, 64, 1024] (replicated)
    ue = np.zeros((NUE,), np.float64)
    for c in range(N_CORES):
        ue += res.results[c]["ueint_out"].reshape(NUE).astype(np.float64)
    out_stacked = np.stack([
        pvnT[0].T.reshape(A, U, ANT),
        pvnT[1].T.reshape(A, U, ANT),
    ]).astype(np.float32)
    ue_int = ue.astype(np.float32).reshape(NUE, 1)
    return (out_stacked, ue_int), res


def kernel(**inputs):
    if not _is_structured(inputs):
        return _reference_numpy(inputs)
    out, _ = _run_device(inputs, trace=False)
    return out
